# revision 9
# baseline (speedup 1.0000x reference)
"""GRU (B=512, T=512, I=32, H=64) + linear head, data-parallel over 8 NeuronCores.

Per core (BL=64), gate-major layout, single batch group. The recurrent
serial chain is minimized via the linearity split
    h' = u + v,   u = z*h (off-chain),   v = (1-z)*n (on-chain)
so W.h' = W.u + W.v accumulates as two matmuls: the u-parts fire during
tanh, and only v's matmul sits on the chain. Per-step chain:
    MM_v(rz) -> sigmoid -> t1=r*ghn -> t2=t1+gxn -> tanh -> v -> ...
Biases are injected by K=1 broadcast matmuls (b (x) ones) into the PSUM
banks (b_in/b_hn) and the sigmoid's per-partition ACT bias (b_rz).
Gate order z (partitions 0-63), r (64-127); the n-path runs at
partitions 64-127, h/u/v at 0-63; ACT ops do the base moves.
x is host-transposed/bf16; bulk matmuls precompute gx for 8 steps per
bank; h=u+v (Pool zc, DVE sums) feeds batched y-projections per 8
steps; y^T bf16 is DMA'd out once; host adds b_lin and untransposes.
"""

import numpy as np
import ml_dtypes
import concourse.bass as bass
import concourse.mybir as mybir
from concourse.tile import TileContext
from concourse.bass_utils import run_bass_kernel_spmd

B, T, I, O, H = 512, 512, 32, 16, 64
NCORES = 8
BL = B // NCORES            # 64 batch rows per core
G = 2                       # interleaved batch groups per core
GN = BL // G                # batch cols per group
XCH = 8                     # steps per bulk-x chunk (one PSUM bank)
XRING = 4                   # x SBUF ring, in chunks
HR = 32                     # h history ring, in steps
YB = 8                      # steps per y-projection block
f32 = mybir.dt.float32
f32r = mybir.dt.float32r
bf16 = mybir.dt.bfloat16
AF = mybir.ActivationFunctionType
ALU = mybir.AluOpType


class _TC(TileContext):
    """TileContext whose instructions never carry >1 sem wait (walrus
    enforces a hard limit; split the excess onto same-engine nops)."""

    def _drain_and_barrier(self, tick_clock, wait_clock):
        super()._drain_and_barrier(tick_clock, wait_clock)
        nc = self.nc
        for fn in nc.m.functions:
            for blk in fn.blocks:
                out = []
                for inst in blk.instructions:
                    si = getattr(inst, "sync_info", None)
                    waits = list(si.on_wait) if si and si.on_wait else []
                    limit = 1
                    if len(waits) > limit:
                        si.on_wait = waits[-limit:]
                        extra = waits[:-limit]
                        for k in range(len(extra)):
                            eng = nc.engines[inst.engine]
                            nop = eng.nop(nofuse=True)
                            cur = nc.cur_bb.bb.instructions
                            assert cur and cur[-1] is nop.ins
                            cur.pop()
                            nop.ins.sync_info = mybir.SyncInfo(
                                on_wait=[extra[k]], on_update=[])
                            out.append(nop.ins)
                    out.append(inst)
                blk.instructions[:] = out


def build_bass(t_steps=T):
    nch = t_steps // XCH
    nc = bass.Bass("TRN2", target_bir_lowering=False, debug=False,
                   num_devices=NCORES)
    x_d = nc.dram_tensor("x", [I, t_steps * BL], bf16, kind="ExternalInput")
    wxrz_d = nc.dram_tensor("w_xrz", [I, 2 * H], bf16, kind="ExternalInput")
    wxn_d = nc.dram_tensor("w_xn", [I, H], bf16, kind="ExternalInput")
    wrz_d = nc.dram_tensor("w_rz", [H, 2 * H], bf16, kind="ExternalInput")
    wn_d = nc.dram_tensor("w_n", [H, H], bf16, kind="ExternalInput")
    wy_d = nc.dram_tensor("w_y", [H, O], bf16, kind="ExternalInput")
    brz_d = nc.dram_tensor("b_rz", [2 * H, 1], f32, kind="ExternalInput")
    bnrow_d = nc.dram_tensor("b_nrow", [1, 2 * H], bf16, kind="ExternalInput")
    y_d = nc.dram_tensor("y", [O, t_steps * BL], bf16, kind="ExternalOutput")

    with _TC(nc) as tc:
        with (
            tc.tile_pool(name="const", bufs=1) as cpool,
            tc.tile_pool(name="state", bufs=1) as spool,
            tc.tile_pool(name="work", bufs=2) as wpool,
            tc.tile_pool(name="psum", bufs=1, space="PSUM") as ppool,
        ):
            w_xrz = cpool.tile([I, 2 * H], bf16)
            nc.sync.dma_start(w_xrz[:, :], wxrz_d[:, :])
            w_xn = cpool.tile([I, H], bf16)
            nc.sync.dma_start(w_xn[:, :], wxn_d[:, :])
            w_rz = cpool.tile([H, 2 * H], bf16)
            nc.sync.dma_start(w_rz[:, :], wrz_d[:, :])
            w_n = cpool.tile([H, H], bf16)
            nc.sync.dma_start(w_n[:, :], wn_d[:, :])
            w_y = cpool.tile([H, O], bf16)
            nc.sync.dma_start(w_y[:, :], wy_d[:, :])
            b_rz = cpool.tile([2 * H, 1], f32)
            nc.sync.dma_start(b_rz[:, :], brz_d[:, :])
            # row vector [1, 128]: cols 0-63 = b_in, 64-127 = b_hn
            b_nrow = cpool.tile([1, 2 * H], bf16)
            nc.sync.dma_start(b_nrow[:, :], bnrow_d[:, :])
            ones = cpool.tile([1, XCH * BL], bf16)
            nc.vector.memset(ones[:, :], 1.0)

            xr = spool.tile([I, XRING * XCH * BL], bf16)
            hh = spool.tile([H, HR * BL], bf16)
            uu = spool.tile([H, 2 * BL], bf16)   # u ring (2 slots)
            vv = spool.tile([H, 2 * BL], bf16)   # v ring
            nc.vector.memset(uu[:, :], 0.0)
            nc.vector.memset(vv[:, :], 0.0)
            ysb = spool.tile([O, t_steps * BL], bf16)

            CW = XCH * BL  # columns per x chunk

            def dma_x(ch):
                s = (ch % XRING) * CW
                nc.sync.dma_start(xr[:, s:s + CW], x_d[:, ch * CW:(ch + 1) * CW])

            def bulk_x(ch):
                """Preload gx for steps [ch*XCH, (ch+1)*XCH).
                rz-bank [128,512] = gx_rz (+ later W_rz.u + W_rz.v).
                gx-bank rows 64-127 = b_in + gx_n.
                gh-bank rows 64-127 = b_hn (+ later W_n.u + W_n.v)."""
                s = (ch % XRING) * CW
                rzb = ppool.tile([2 * H, CW], f32, tag="gxrz", bufs=2,
                                 name=f"gxrz_{ch}")
                nc.tensor.matmul(rzb[:, :], w_xrz[:, :], xr[:, s:s + CW],
                                 start=True, stop=False)
                gxb = ppool.tile([2 * H, CW], f32, tag="gxn", bufs=2,
                                 name=f"gxn_{ch}")
                nc.tensor.matmul(gxb[H:2 * H, :], b_nrow[:, 0:H],
                                 ones[:, :], start=True, stop=False)
                nc.tensor.matmul(gxb[H:2 * H, :], w_xn[:, :], xr[:, s:s + CW],
                                 start=False, stop=True)
                ghb = ppool.tile([2 * H, CW], f32, tag="ghn", bufs=2,
                                 name=f"ghn_{ch}")
                nc.tensor.matmul(ghb[H:2 * H, :], b_nrow[:, H:2 * H],
                                 ones[:, :], start=True, stop=False)
                return rzb, gxb, ghb

            def mm_u(bk, t):
                """W.u(t-1) contributions for step t (fire during tanh(t-1))."""
                us = ((t - 1) % 2) * BL
                cols = (t % XCH) * BL
                rzb, gxb, ghb = bk
                nc.tensor.matmul(rzb[:, cols:cols + BL], w_rz[:, :],
                                 uu[:, us:us + BL], start=False, stop=False,
                                 skip_group_check=True)
                nc.tensor.matmul(ghb[H:2 * H, cols:cols + BL], w_n[:, :],
                                 uu[:, us:us + BL], start=False, stop=False,
                                 skip_group_check=True)

            # prologue
            for ch in range(min(3, nch)):
                dma_x(ch)
            bk_cur = bulk_x(0)
            bk_nxt = bulk_x(1) if nch > 1 else None
            mm_u(bk_cur, 0)  # u(-1) = 0

            for t in range(t_steps):
                k, tl = divmod(t, XCH)
                if tl == 0 and k + 3 < nch:
                    dma_x(k + 3)
                if tl == 4 and k + 2 < nch:
                    bk_nxt2 = bulk_x(k + 2)

                cols = tl * BL
                vs = ((t - 1) % 2) * BL   # v(t-1) slot
                us = (t % 2) * BL         # u(t) slot
                hcol = (t % HR) * BL
                pcol = ((t - 1) % HR) * BL
                rzb, gxb, ghb = bk_cur

                # --- PE: v(t-1) contributions (chain head)
                nc.tensor.matmul(rzb[:, cols:cols + BL], w_rz[:, :],
                                 vv[:, vs:vs + BL], start=False, stop=True,
                                 skip_group_check=True)
                nc.tensor.matmul(ghb[H:2 * H, cols:cols + BL], w_n[:, :],
                                 vv[:, vs:vs + BL], start=False, stop=True,
                                 skip_group_check=True)

                # --- ACT: sigma (z at partitions 0-63, r at 64-127)
                rz = wpool.tile([2 * H, BL], f32, tag="rz", bufs=2,
                                name=f"rz_{t}")
                nc.scalar.activation(rz[:, :], rzb[:, cols:cols + BL],
                                     AF.Sigmoid, bias=b_rz[:, 0:1])

                # --- Pool: h(t-1) = u(t-1) + v(t-1) (off-chain, bf16)
                if t > 0:
                    nc.gpsimd.tensor_tensor(
                        hh[:, pcol:pcol + BL], uu[:, vs:vs + BL],
                        vv[:, vs:vs + BL], ALU.add)
                else:
                    nc.vector.memset(hh[:, pcol:pcol + BL], 0.0)

                # --- DVE: n-path at partitions 64-127
                t1 = wpool.tile([2 * H, BL], f32, tag="t1", bufs=2,
                                name=f"t1_{t}")
                nc.vector.tensor_tensor(t1[H:2 * H, :], rz[H:2 * H, :],
                                        ghb[H:2 * H, cols:cols + BL],
                                        ALU.mult)
                t2 = wpool.tile([2 * H, BL], f32, tag="t2", bufs=2,
                                name=f"t2_{t}")
                nc.vector.tensor_tensor(t2[H:2 * H, :], t1[H:2 * H, :],
                                        gxb[H:2 * H, cols:cols + BL],
                                        ALU.add)

                # --- Pool: zc = 1 - z (off-chain, ready before v)
                zc = wpool.tile([H, BL], f32, tag="zc", bufs=2,
                                name=f"zc_{t}")
                nc.gpsimd.tensor_scalar(zc[:, :], rz[0:H, :], -1.0, 1.0,
                                        ALU.mult, ALU.add)

                # --- DVE: u(t) = z*h(t-1) (fills the DVE gap before tanh)
                nc.vector.tensor_tensor(uu[:, us:us + BL], rz[0:H, :],
                                        hh[:, pcol:pcol + BL], ALU.mult)

                # --- ACT: n = tanh(t2), to partitions 0-63 (biases in bank)
                n_sb = wpool.tile([H, BL], f32, tag="n", bufs=2,
                                  name=f"n_{t}")
                nc.scalar.activation(n_sb[:, :], t2[H:2 * H, :], AF.Tanh)

                # --- DVE: v(t) = zc*n (chain tail)
                nc.vector.tensor_tensor(vv[:, us:us + BL], zc[:, :],
                                        n_sb[:, :], ALU.mult)

                # --- PE: u(t) contributions for step t+1 (during tanh window)
                if t + 1 < t_steps:
                    bk = bk_cur if tl + 1 < XCH else bk_nxt
                    mm_u(bk, t + 1)

                # --- PE/ACT: batched y projection per YB steps (h(t-YB..t-1))
                if t % YB == YB - 1 and t >= YB:
                    blk = t // YB - 1
                    rs = ((blk * YB) % HR) * BL
                    yp = ppool.tile([O, YB * BL], f32, tag="y", bufs=2,
                                    name=f"y_{blk}")
                    nc.tensor.matmul(yp[:, :], w_y[:, :],
                                     hh[:, rs:rs + YB * BL],
                                     start=True, stop=True)
                    nc.scalar.activation(
                        ysb[:, blk * YB * BL:(blk + 1) * YB * BL],
                        yp[:, :], AF.Copy)

                if tl == XCH - 1:
                    bk_cur = bk_nxt
                    if k + 2 < nch:
                        bk_nxt = bk_nxt2

            # epilogue: h(T-1) + the last two y blocks
            vs = ((t_steps - 1) % 2) * BL
            pcol = ((t_steps - 1) % HR) * BL
            nc.gpsimd.tensor_tensor(hh[:, pcol:pcol + BL],
                                    uu[:, vs:vs + BL], vv[:, vs:vs + BL],
                                    ALU.add)
            for blk in (t_steps // YB - 1,):
                rs = ((blk * YB) % HR) * BL
                yp = ppool.tile([O, YB * BL], f32, tag="y", bufs=2,
                                name=f"y_{blk}")
                nc.tensor.matmul(yp[:, :], w_y[:, :], hh[:, rs:rs + YB * BL],
                                 start=True, stop=True)
                nc.scalar.activation(
                    ysb[:, blk * YB * BL:(blk + 1) * YB * BL],
                    yp[:, :], AF.Copy)
            nc.sync.dma_start(y_d[:, :], ysb[:, :])
    return nc


def prep_consts(W_ih, W_hh, b_ih, b_hh, W_lin, b_lin):
    W_ih = np.asarray(W_ih, np.float32)
    W_hh = np.asarray(W_hh, np.float32)
    b_ih = np.asarray(b_ih, np.float32)
    b_hh = np.asarray(b_hh, np.float32)
    # gate order z (0-63), r (64-127), n via separate tensors
    zr = list(range(H, 2 * H)) + list(range(0, H))
    bf = ml_dtypes.bfloat16
    pad = np.zeros((H,), np.float32)
    return {
        "w_xrz": np.ascontiguousarray(W_ih[zr].T).astype(bf),
        "w_xn": np.ascontiguousarray(W_ih[2 * H:3 * H].T).astype(bf),
        "w_rz": np.ascontiguousarray(W_hh[zr].T).astype(bf),
        "w_n": np.ascontiguousarray(W_hh[2 * H:3 * H].T).astype(bf),
        "w_y": np.ascontiguousarray(np.asarray(W_lin, np.float32).T).astype(bf),
        "b_rz": np.ascontiguousarray(
            (b_ih + b_hh)[zr].reshape(2 * H, 1)),
        "b_nrow": np.ascontiguousarray(np.concatenate(
            [b_ih[2 * H:3 * H], b_hh[2 * H:3 * H]]).reshape(1, 2 * H)).astype(bf),
    }


_cached = {}


def build_in_maps(np_inputs):
    x = np.asarray(np_inputs["x"], np.float32)
    consts = prep_consts(np_inputs["W_ih"], np_inputs["W_hh"],
                         np_inputs["b_ih"], np_inputs["b_hh"],
                         np_inputs["W_lin"], np_inputs["b_lin"])
    in_maps = []
    for cid in range(NCORES):
        m = dict(consts)
        xc = x[cid * BL:(cid + 1) * BL]           # [BL, T, I]
        m["x"] = np.ascontiguousarray(
            xc.transpose(2, 1, 0).reshape(I, T * BL)).astype(ml_dtypes.bfloat16)
        in_maps.append(m)
    return in_maps


def kernel(x, W_ih, W_hh, b_ih, b_hh, W_lin, b_lin):
    if "nc" not in _cached:
        _cached["nc"] = build_bass()
    nc = _cached["nc"]
    in_maps = build_in_maps(dict(x=x, W_ih=W_ih, W_hh=W_hh, b_ih=b_ih,
                                 b_hh=b_hh, W_lin=W_lin, b_lin=b_lin))
    res = run_bass_kernel_spmd(nc, in_maps, core_ids=list(range(NCORES)))
    b_lin = np.asarray(b_lin, np.float32)
    outs = []
    for cid in range(NCORES):
        yT = np.asarray(res.results[cid]["y"]).astype(np.float32)
        y = yT.reshape(O, T, BL) + b_lin[:, None, None]
        outs.append(y.transpose(2, 1, 0))          # [BL, T, O]
    return np.ascontiguousarray(np.concatenate(outs, 0))


# revision 10
# speedup vs baseline: 1.1992x; 1.1992x over previous
"""GRU (B=512, T=512, I=32, H=64) + linear head, data-parallel over 8 NeuronCores.

Per core (BL=64), gate-major layout, single batch group. The recurrent
serial chain is minimized via the linearity split
    h' = u + v,   u = z*h (off-chain),   v = (1-z)*n (on-chain)
so W.h' = W.u + W.v accumulates as two matmuls: the u-parts fire during
tanh, and only v's matmul sits on the chain. Per-step chain:
    MM_v(rz) -> sigmoid -> t1=r*ghn -> t2=t1+gxn -> tanh -> v -> ...
Biases are injected by K=1 broadcast matmuls (b (x) ones) into the PSUM
banks (b_in/b_hn) and the sigmoid's per-partition ACT bias (b_rz).
Gate order z (partitions 0-63), r (64-127); the n-path runs at
partitions 64-127, h/u/v at 0-63; ACT ops do the base moves.
x is host-transposed/bf16; bulk matmuls precompute gx for 8 steps per
bank; h=u+v (Pool zc, DVE sums) feeds batched y-projections per 8
steps; y^T bf16 is DMA'd out once; host adds b_lin and untransposes.
"""

import numpy as np
import ml_dtypes
import concourse.bass as bass
import concourse.mybir as mybir
from concourse.tile import TileContext
from concourse.bass_utils import run_bass_kernel_spmd

B, T, I, O, H = 512, 512, 32, 16, 64
NCORES = 8
BL = B // NCORES            # 64 batch rows per core
G = 2                       # interleaved batch groups per core
GN = BL // G                # batch cols per group
XCH = 8                     # steps per bulk-x chunk (one PSUM bank)
XRING = 4                   # x SBUF ring, in chunks
HR = 32                     # h history ring, in steps
YB = 8                      # steps per y-projection block
f32 = mybir.dt.float32
f32r = mybir.dt.float32r
bf16 = mybir.dt.bfloat16
AF = mybir.ActivationFunctionType
ALU = mybir.AluOpType


class _TC(TileContext):
    """TileContext whose instructions never carry >1 sem wait (walrus
    enforces a hard limit; split the excess onto same-engine nops)."""

    def _drain_and_barrier(self, tick_clock, wait_clock):
        super()._drain_and_barrier(tick_clock, wait_clock)
        nc = self.nc
        for fn in nc.m.functions:
            for blk in fn.blocks:
                out = []
                for inst in blk.instructions:
                    si = getattr(inst, "sync_info", None)
                    waits = list(si.on_wait) if si and si.on_wait else []
                    limit = 1
                    if len(waits) > limit:
                        si.on_wait = waits[-limit:]
                        extra = waits[:-limit]
                        for k in range(len(extra)):
                            eng = nc.engines[inst.engine]
                            nop = eng.nop(nofuse=True)
                            cur = nc.cur_bb.bb.instructions
                            assert cur and cur[-1] is nop.ins
                            cur.pop()
                            nop.ins.sync_info = mybir.SyncInfo(
                                on_wait=[extra[k]], on_update=[])
                            out.append(nop.ins)
                    out.append(inst)
                blk.instructions[:] = out


def build_bass(t_steps=T):
    nch = t_steps // XCH
    nc = bass.Bass("TRN2", target_bir_lowering=False, debug=False,
                   num_devices=NCORES)
    x_d = nc.dram_tensor("x", [I, t_steps * BL], bf16, kind="ExternalInput")
    wxrz_d = nc.dram_tensor("w_xrz", [I, 2 * H], bf16, kind="ExternalInput")
    wxn_d = nc.dram_tensor("w_xn", [I, H], bf16, kind="ExternalInput")
    wrz_d = nc.dram_tensor("w_rz", [H, 2 * H], bf16, kind="ExternalInput")
    wn_d = nc.dram_tensor("w_n", [H, H], bf16, kind="ExternalInput")
    wy_d = nc.dram_tensor("w_y", [H, O], bf16, kind="ExternalInput")
    brz_d = nc.dram_tensor("b_rz", [2 * H, 1], f32, kind="ExternalInput")
    bnrow_d = nc.dram_tensor("b_nrow", [1, 2 * H], bf16, kind="ExternalInput")
    y_d = nc.dram_tensor("y", [O, t_steps * BL], bf16, kind="ExternalOutput")

    with _TC(nc) as tc:
        with (
            tc.tile_pool(name="const", bufs=1) as cpool,
            tc.tile_pool(name="state", bufs=1) as spool,
            tc.tile_pool(name="work", bufs=2) as wpool,
            tc.tile_pool(name="psum", bufs=1, space="PSUM") as ppool,
        ):
            w_xrz = cpool.tile([I, 2 * H], bf16)
            nc.sync.dma_start(w_xrz[:, :], wxrz_d[:, :])
            w_xn = cpool.tile([I, H], bf16)
            nc.sync.dma_start(w_xn[:, :], wxn_d[:, :])
            w_rz = cpool.tile([H, 2 * H], bf16)
            nc.sync.dma_start(w_rz[:, :], wrz_d[:, :])
            w_n = cpool.tile([H, H], bf16)
            nc.sync.dma_start(w_n[:, :], wn_d[:, :])
            w_y = cpool.tile([H, O], bf16)
            nc.sync.dma_start(w_y[:, :], wy_d[:, :])
            b_rz = cpool.tile([2 * H, 1], f32)
            nc.sync.dma_start(b_rz[:, :], brz_d[:, :])
            # row vector [1, 128]: cols 0-63 = b_in, 64-127 = b_hn
            b_nrow = cpool.tile([1, 2 * H], bf16)
            nc.sync.dma_start(b_nrow[:, :], bnrow_d[:, :])
            ones = cpool.tile([1, XCH * BL], bf16)
            nc.vector.memset(ones[:, :], 1.0)

            xr = spool.tile([I, XRING * XCH * BL], bf16)
            hh = spool.tile([H, HR * BL], bf16)
            uu = spool.tile([H, 2 * BL], bf16)   # u ring (2 slots)
            vv = spool.tile([H, 2 * BL], bf16)   # v ring
            nc.vector.memset(uu[:, :], 0.0)
            nc.vector.memset(vv[:, :], 0.0)
            ysb = spool.tile([O, t_steps * BL], bf16)

            CW = XCH * BL  # columns per x chunk

            def dma_x(ch):
                s = (ch % XRING) * CW
                nc.sync.dma_start(xr[:, s:s + CW], x_d[:, ch * CW:(ch + 1) * CW])

            def bulk_x(ch):
                """Preload gx for steps [ch*XCH, (ch+1)*XCH).
                rz-bank [128,512] = gx_rz (+ later W_rz.u + W_rz.v).
                gx-bank rows 64-127 = b_in + gx_n.
                gh-bank rows 64-127 = b_hn (+ later W_n.u + W_n.v)."""
                s = (ch % XRING) * CW
                rzb = ppool.tile([2 * H, CW], f32, tag="gxrz", bufs=2,
                                 name=f"gxrz_{ch}")
                nc.tensor.matmul(rzb[:, :], w_xrz[:, :], xr[:, s:s + CW],
                                 start=True, stop=False)
                gxb = ppool.tile([2 * H, CW], f32, tag="gxn", bufs=2,
                                 name=f"gxn_{ch}")
                nc.tensor.matmul(gxb[H:2 * H, :], b_nrow[:, 0:H],
                                 ones[:, :], start=True, stop=False)
                nc.tensor.matmul(gxb[H:2 * H, :], w_xn[:, :], xr[:, s:s + CW],
                                 start=False, stop=True)
                ghb = ppool.tile([2 * H, CW], f32, tag="ghn", bufs=2,
                                 name=f"ghn_{ch}")
                nc.tensor.matmul(ghb[H:2 * H, :], b_nrow[:, H:2 * H],
                                 ones[:, :], start=True, stop=False)
                return rzb, gxb, ghb

            def mm_u(bk, t):
                """W.u(t-1) contributions for step t (fire during tanh(t-1))."""
                us = ((t - 1) % 2) * BL
                cols = (t % XCH) * BL
                rzb, gxb, ghb = bk
                nc.tensor.matmul(rzb[:, cols:cols + BL], w_rz[:, :],
                                 uu[:, us:us + BL], start=False, stop=False,
                                 skip_group_check=True)
                nc.tensor.matmul(ghb[H:2 * H, cols:cols + BL], w_n[:, :],
                                 uu[:, us:us + BL], start=False, stop=False,
                                 skip_group_check=True)

            # prologue
            for ch in range(min(3, nch)):
                dma_x(ch)
            bk_cur = bulk_x(0)
            bk_nxt = bulk_x(1) if nch > 1 else None
            mm_u(bk_cur, 0)  # u(-1) = 0

            for t in range(t_steps):
                k, tl = divmod(t, XCH)
                if tl == 0 and k + 3 < nch:
                    dma_x(k + 3)

                cols = tl * BL
                vs = ((t - 1) % 2) * BL   # v(t-1) slot
                us = (t % 2) * BL         # u(t) slot
                hcol = (t % HR) * BL
                pcol = ((t - 1) % HR) * BL
                rzb, gxb, ghb = bk_cur

                # --- PE: v(t-1) contributions (chain head)
                nc.tensor.matmul(rzb[:, cols:cols + BL], w_rz[:, :],
                                 vv[:, vs:vs + BL], start=False, stop=True,
                                 skip_group_check=True)
                nc.tensor.matmul(ghb[H:2 * H, cols:cols + BL], w_n[:, :],
                                 vv[:, vs:vs + BL], start=False, stop=True,
                                 skip_group_check=True)

                # --- ACT: sigma (z at partitions 0-63, r at 64-127)
                rz = wpool.tile([2 * H, BL], f32, tag="rz", bufs=2,
                                name=f"rz_{t}")
                nc.scalar.activation(rz[:, :], rzb[:, cols:cols + BL],
                                     AF.Sigmoid, bias=b_rz[:, 0:1])

                # --- Pool: h(t-1) = u(t-1) + v(t-1) (off-chain, bf16)
                if t > 0:
                    nc.gpsimd.tensor_tensor(
                        hh[:, pcol:pcol + BL], uu[:, vs:vs + BL],
                        vv[:, vs:vs + BL], ALU.add)
                else:
                    nc.vector.memset(hh[:, pcol:pcol + BL], 0.0)

                # --- DVE: n-path at partitions 64-127
                t1 = wpool.tile([2 * H, BL], f32, tag="t1", bufs=2,
                                name=f"t1_{t}")
                nc.vector.tensor_tensor(t1[H:2 * H, :], rz[H:2 * H, :],
                                        ghb[H:2 * H, cols:cols + BL],
                                        ALU.mult)
                t2 = wpool.tile([2 * H, BL], f32, tag="t2", bufs=2,
                                name=f"t2_{t}")
                nc.vector.tensor_tensor(t2[H:2 * H, :], t1[H:2 * H, :],
                                        gxb[H:2 * H, cols:cols + BL],
                                        ALU.add)

                # --- Pool: zc = 1 - z (off-chain, ready before v)
                zc = wpool.tile([H, BL], f32, tag="zc", bufs=2,
                                name=f"zc_{t}")
                nc.gpsimd.tensor_scalar(zc[:, :], rz[0:H, :], -1.0, 1.0,
                                        ALU.mult, ALU.add)

                # --- DVE: u(t) = z*h(t-1) (fills the DVE gap before tanh)
                nc.vector.tensor_tensor(uu[:, us:us + BL], rz[0:H, :],
                                        hh[:, pcol:pcol + BL], ALU.mult)

                # --- ACT: n = tanh(t2), to partitions 0-63 (biases in bank)
                n_sb = wpool.tile([H, BL], f32, tag="n", bufs=2,
                                  name=f"n_{t}")
                nc.scalar.activation(n_sb[:, :], t2[H:2 * H, :], AF.Tanh)

                # --- DVE: v(t) = zc*n (chain tail)
                nc.vector.tensor_tensor(vv[:, us:us + BL], zc[:, :],
                                        n_sb[:, :], ALU.mult)

                # --- PE: u(t) contributions for step t+1 (during tanh window)
                if t + 1 < t_steps:
                    bk = bk_cur if tl + 1 < XCH else bk_nxt
                    mm_u(bk, t + 1)
                if tl == 4 and k + 2 < nch:
                    bk_nxt2 = bulk_x(k + 2)

                # --- PE/ACT: batched y projection per YB steps (h(t-YB..t-1))
                if t % YB == YB - 1 and t >= YB:
                    blk = t // YB - 1
                    rs = ((blk * YB) % HR) * BL
                    yp = ppool.tile([O, YB * BL], f32, tag="y", bufs=2,
                                    name=f"y_{blk}")
                    nc.tensor.matmul(yp[:, :], w_y[:, :],
                                     hh[:, rs:rs + YB * BL],
                                     start=True, stop=True)
                    nc.scalar.activation(
                        ysb[:, blk * YB * BL:(blk + 1) * YB * BL],
                        yp[:, :], AF.Copy)

                if tl == XCH - 1:
                    bk_cur = bk_nxt
                    if k + 2 < nch:
                        bk_nxt = bk_nxt2

            # epilogue: h(T-1) + the last two y blocks
            vs = ((t_steps - 1) % 2) * BL
            pcol = ((t_steps - 1) % HR) * BL
            nc.gpsimd.tensor_tensor(hh[:, pcol:pcol + BL],
                                    uu[:, vs:vs + BL], vv[:, vs:vs + BL],
                                    ALU.add)
            for blk in (t_steps // YB - 1,):
                rs = ((blk * YB) % HR) * BL
                yp = ppool.tile([O, YB * BL], f32, tag="y", bufs=2,
                                name=f"y_{blk}")
                nc.tensor.matmul(yp[:, :], w_y[:, :], hh[:, rs:rs + YB * BL],
                                 start=True, stop=True)
                nc.scalar.activation(
                    ysb[:, blk * YB * BL:(blk + 1) * YB * BL],
                    yp[:, :], AF.Copy)
            nc.sync.dma_start(y_d[:, :], ysb[:, :])
    return nc


def prep_consts(W_ih, W_hh, b_ih, b_hh, W_lin, b_lin):
    W_ih = np.asarray(W_ih, np.float32)
    W_hh = np.asarray(W_hh, np.float32)
    b_ih = np.asarray(b_ih, np.float32)
    b_hh = np.asarray(b_hh, np.float32)
    # gate order z (0-63), r (64-127), n via separate tensors
    zr = list(range(H, 2 * H)) + list(range(0, H))
    bf = ml_dtypes.bfloat16
    pad = np.zeros((H,), np.float32)
    return {
        "w_xrz": np.ascontiguousarray(W_ih[zr].T).astype(bf),
        "w_xn": np.ascontiguousarray(W_ih[2 * H:3 * H].T).astype(bf),
        "w_rz": np.ascontiguousarray(W_hh[zr].T).astype(bf),
        "w_n": np.ascontiguousarray(W_hh[2 * H:3 * H].T).astype(bf),
        "w_y": np.ascontiguousarray(np.asarray(W_lin, np.float32).T).astype(bf),
        "b_rz": np.ascontiguousarray(
            (b_ih + b_hh)[zr].reshape(2 * H, 1)),
        "b_nrow": np.ascontiguousarray(np.concatenate(
            [b_ih[2 * H:3 * H], b_hh[2 * H:3 * H]]).reshape(1, 2 * H)).astype(bf),
    }


_cached = {}


def build_in_maps(np_inputs):
    x = np.asarray(np_inputs["x"], np.float32)
    consts = prep_consts(np_inputs["W_ih"], np_inputs["W_hh"],
                         np_inputs["b_ih"], np_inputs["b_hh"],
                         np_inputs["W_lin"], np_inputs["b_lin"])
    in_maps = []
    for cid in range(NCORES):
        m = dict(consts)
        xc = x[cid * BL:(cid + 1) * BL]           # [BL, T, I]
        m["x"] = np.ascontiguousarray(
            xc.transpose(2, 1, 0).reshape(I, T * BL)).astype(ml_dtypes.bfloat16)
        in_maps.append(m)
    return in_maps


def kernel(x, W_ih, W_hh, b_ih, b_hh, W_lin, b_lin):
    if "nc" not in _cached:
        _cached["nc"] = build_bass()
    nc = _cached["nc"]
    in_maps = build_in_maps(dict(x=x, W_ih=W_ih, W_hh=W_hh, b_ih=b_ih,
                                 b_hh=b_hh, W_lin=W_lin, b_lin=b_lin))
    res = run_bass_kernel_spmd(nc, in_maps, core_ids=list(range(NCORES)))
    b_lin = np.asarray(b_lin, np.float32)
    outs = []
    for cid in range(NCORES):
        yT = np.asarray(res.results[cid]["y"]).astype(np.float32)
        y = yT.reshape(O, T, BL) + b_lin[:, None, None]
        outs.append(y.transpose(2, 1, 0))          # [BL, T, O]
    return np.ascontiguousarray(np.concatenate(outs, 0))
